# revision 1
# baseline (speedup 1.0000x reference)
"""Trainium2 Bass kernel for nn_DSVDD_9972914061970 (retrieval_knn).

Per-core computation (one batch image per NeuronCore):
  phi = pool3(W1*p1) + up2(pool3(W2*p2)) + up4(pool3(W3*p3)) + K0   [448, 3136]
  rank[s,m] = 2*phi.C - |C|^2          (augmented K=450 matmul, bf16)
  top-3 largest of rank -> d_i = sqrt(|phi_s|^2 - v_i) -> softmin score

Engine placement:
  PE : channel matmuls (w-pool fused as 3 shifted PSUM accums), up4+K0 as a
       matmul against a host kron operator, |phi|^2 ones-matmul, cdist.
  DVE: h-pools, 2x bilinear upsample, max8 top-8 extraction from PSUM.
  ACT: PSUM->SBUF bf16 casts, squares, sqrt/exp of the score stage.
"""
import numpy as np
import ml_dtypes

import concourse.bacc as bacc
import concourse.mybir as mybir
from concourse.tile import TileContext

BF16 = ml_dtypes.bfloat16
F32 = mybir.dt.float32
BF = mybir.dt.bfloat16
AF = mybir.ActivationFunctionType
OP = mybir.AluOpType

CH = 448          # output channels
HW = 3136         # 56*56
NRT = 25          # row tiles of 128 spatial positions (24*128 + 64)
PIECES = [(0, 1024), (1024, 1024), (2048, 1088)]  # cdist psum pieces

STAGES = frozenset('AFCS')
USE_GPSIMD = True  # build stages: A=descriptor F=features C=cdist S=score

R3_PHASES = [  # unused in v2 (kept for reference): 4x tap table
    (0, -1, 0.375, 0.625), (1, -1, 0.125, 0.875),
    (2, +1, 0.125, 0.875), (3, +1, 0.375, 0.625),
]


def resize_matrix(n_in, n_out):
    """jax.image.resize 'bilinear' 1D operator [n_out, n_in] (verified exact)."""
    M = np.zeros((n_out, n_in), np.float32)
    s = n_in / n_out
    for j in range(n_out):
        pos = (j + 0.5) * s - 0.5
        i0 = int(np.floor(pos)); f = pos - i0
        ws = {}
        for k, wt in [(i0, 1 - f), (i0 + 1, f)]:
            if 0 <= k < n_in:
                ws[k] = ws.get(k, 0) + wt
        t = sum(ws.values())
        for k, wt in ws.items():
            M[j, k] = wt / t
    return M


# ---------------------------------------------------------------- host side
def host_prep_shared(W, b, C):
    """Batch-independent packed arrays (bf16)."""
    W = np.asarray(W, np.float32)
    b = np.asarray(b, np.float32)
    C = np.asarray(C, np.float32)
    w1 = (W[:, :256] / 9.0).T.copy()      # [256, 448]
    w2 = (W[:, 256:768] / 9.0).T.copy()   # [512, 448]
    w3 = (W[:, 768:1792] / 9.0).T.copy()  # [1024, 448]

    def chunk_k(wt, n):  # [n*128, 448] -> [128, n, 448]
        return np.ascontiguousarray(
            wt.reshape(n, 128, CH).transpose(1, 0, 2)).astype(BF16)

    w1t = chunk_k(w1, 2)
    w2t = chunk_k(w2, 4)
    w3t = chunk_k(w3, 8)

    # up4 operator (applied to the already-pooled 14x14 maps) + K0 coord rows
    h = w = 56
    xx = (np.arange(h, dtype=np.float32) / (h - 1)) * 2.0 - 1.0
    yy = (np.arange(w, dtype=np.float32) / (w - 1)) * 2.0 - 1.0
    xx_f = np.broadcast_to(xx[:, None], (h, w)).reshape(-1)
    yy_f = np.broadcast_to(yy[None, :], (h, w)).reshape(-1)
    M4 = resize_matrix(14, 56)
    S3 = np.kron(M4, M4).T.astype(np.float32)      # [196, 3136]
    s3a = S3[0:98].astype(BF16)                    # [98, 3136]
    s3b = np.zeros((101, HW), np.float32)
    s3b[0:98] = S3[98:196]
    s3b[98] = 1.0           # pairs with lhsT row b
    s3b[99] = xx_f          # pairs with lhsT row W_xx
    s3b[100] = yy_f         # pairs with lhsT row W_yy
    s3b = s3b.astype(BF16)
    # lhsT coordinate rows [3, 448]: b, W_xx, W_yy   (K0 = b + Wxx*xx + Wyy*yy)
    coefw = np.stack([b, W[:, 1792], W[:, 1793]]).astype(BF16)

    centers = (C * C).sum(axis=0)                  # [3136] f32
    chi = centers.astype(BF16).astype(np.float32)
    clo = centers - chi
    C2 = (2.0 * C).astype(BF16)                    # [448, 3136]
    rhs012 = np.ascontiguousarray(
        C2[:336].reshape(3, 112, HW).transpose(1, 0, 2))       # [112, 3, 3136]
    rhs3 = np.zeros((114, HW), BF16)
    rhs3[:112] = C2[336:448]
    rhs3[112] = (-chi).astype(BF16)
    rhs3[113] = (-clo).astype(BF16)
    ones2 = np.ones((2, HW), BF16)                 # phi3 "ones" aug rows
    return dict(w1t=w1t, w2t=w2t, w3t=w3t, s3a=s3a, s3b=s3b, coefw=coefw,
                rhs012=rhs012, rhs3=rhs3, ones2=ones2)


def host_prep_core(p1, p2, p3):
    """Per-core packed activations (bf16, zero-padded along w for the PE pool)."""
    def pad_pack(p, c, hh, ww, n):
        a = np.asarray(p, np.float32).reshape(c, hh, ww)
        ap = np.zeros((c, hh, ww + 2), np.float32)
        ap[:, :, 1:ww + 1] = a
        ap = ap.reshape(n, 128, hh * (ww + 2)).transpose(1, 0, 2)
        return np.ascontiguousarray(ap).astype(BF16)
    return dict(
        p1pad=pad_pack(p1, 256, 56, 56, 2),    # [128, 2, 3248]
        p2pad=pad_pack(p2, 512, 28, 28, 4),    # [128, 4, 840]
        p3pad=pad_pack(p3, 1024, 14, 14, 8),   # [128, 8, 224]
    )


def host_postprocess(score_2d):
    """[128, 25] per-core -> [56, 56]."""
    return np.asarray(score_2d).T.reshape(-1)[:HW].reshape(56, 56)


# ---------------------------------------------------------------- bass build
def build_nc(n_cores=8):
    nc = bacc.Bacc("TRN2", target_bir_lowering=False, debug=False,
                   num_devices=n_cores)
    dt = {}
    def din(name, shape, dtype=BF):
        dt[name] = nc.dram_tensor(name, shape, dtype, kind="ExternalInput")
    din("p1pad", [128, 2, 3248]); din("p2pad", [128, 4, 840]); din("p3pad", [128, 8, 224])
    din("w1t", [128, 2, 448]);    din("w2t", [128, 4, 448]);   din("w3t", [128, 8, 448])
    din("s3a", [98, HW]);         din("s3b", [101, HW]);       din("coefw", [3, 448])
    din("rhs012", [112, 3, HW]);  din("rhs3", [114, HW]);      din("ones2", [2, HW])
    score_d = nc.dram_tensor("score", [128, NRT], F32, kind="ExternalOutput")

    with TileContext(nc) as tc:
        build_body(nc, tc, dt, score_d)
    nc.compile()
    return nc


def build_body(nc, tc, dt, score_d):
    with tc.tile_pool(name="sb", bufs=1) as sb, \
         tc.tile_pool(name="sb1", bufs=1) as sb1, \
         tc.tile_pool(name="sb2", bufs=2) as sb2:

        # ---- persistent SBUF inputs (DMA order = consumption order)
        sp2 = sb.tile([128, 4, 840], BF, tag="sp2")
        sw2 = sb.tile([128, 4, 448], BF, tag="sw2")
        for kc in range(4):   # split so the first A2 matmul starts asap
            nc.sync.dma_start(sw2[:, kc, :], dt["w2t"].ap()[:, kc, :])
            nc.sync.dma_start(sp2[:, kc, :], dt["p2pad"].ap()[:, kc, :])
        sp3 = sb.tile([128, 8, 224], BF, tag="sp3")
        nc.sync.dma_start(sp3[:], dt["p3pad"].ap())
        sw3 = sb.tile([128, 8, 448], BF, tag="sw3")
        nc.sync.dma_start(sw3[:], dt["w3t"].ap())
        s3a = sb.tile([98, HW], BF, tag="s3a")
        nc.sync.dma_start(s3a[:], dt["s3a"].ap())
        s3b = sb.tile([101, HW], BF, tag="s3b")
        nc.sync.dma_start(s3b[:], dt["s3b"].ap())
        sp1 = sb.tile([128, 2, 3248], BF, tag="sp1")
        nc.sync.dma_start(sp1[:], dt["p1pad"].ap())
        sw1 = sb.tile([128, 2, 448], BF, tag="sw1")
        nc.sync.dma_start(sw1[:], dt["w1t"].ap())
        rhs012 = sb.tile([112, 3, HW], BF, tag="rhs012")
        nc.sync.dma_start(rhs012[:], dt["rhs012"].ap())
        rhs3 = sb.tile([114, HW], BF, tag="rhs3")
        nc.sync.dma_start(rhs3[:], dt["rhs3"].ap())
        # phi accumulators + the up4(P3)+K0 staging buffer
        phi012 = sb.tile([112, 3, HW], BF)
        phi3 = sb.tile([114, HW], BF)
        nc.sync.dma_start(phi3[112:114, :], dt["ones2"].ap())
        u3k0 = sb.tile([112, 4, HW], BF)

        def phi_cc(cc):
            return phi012[:, cc, :] if cc < 3 else phi3[0:112, :]

        # ================= stage A: descriptor =================
        with tc.tile_pool(name="psA", bufs=2, space="PSUM") as psA, \
             tc.tile_pool(name="psU", bufs=4, space="PSUM") as psU:
            # A2 + 2x upsample phases early (PE first; elementwise tail on
            # DVE/gpsimd hides under the remaining matmuls)
            ufs = []
            ves = []
            for cc in range(4):
                ccs = slice(cc * 112, (cc + 1) * 112)
                ve = nc.gpsimd if (cc % 2 == 0 and USE_GPSIMD) else nc.vector
                ufs.append(build_a2_r2(nc, sb, sb1, sb2, psA, sp2, sw2, ccs, cc, ve=ve))
                ves.append(ve)

            # p3 3x3 pool on DVE (tiny)
            t3pad = sb.tile([128, 8, 16, 14], BF)
            nc.vector.memset(t3pad[:, :, 0, :], 0.0)
            nc.vector.memset(t3pad[:, :, 15, :], 0.0)
            sp3v = sp3[:].rearrange("p a (h w) -> p a h w", w=16)  # [128,8,14,16]
            wtmp = sb.tile([128, 8, 14, 14], BF)
            nc.vector.tensor_tensor(wtmp[:], sp3v[:, :, :, 0:14], sp3v[:, :, :, 2:16], op=OP.add)
            nc.vector.tensor_tensor(t3pad[:, :, 1:15, :], wtmp[:], sp3v[:, :, :, 1:15], op=OP.add)
            htmp = sb.tile([128, 8, 14, 14], BF)
            nc.vector.tensor_tensor(htmp[:], t3pad[:, :, 0:14, :], t3pad[:, :, 2:16, :], op=OP.add)
            p3c = sb.tile([128, 8, 14, 14], BF)
            nc.vector.tensor_tensor(p3c[:], htmp[:], t3pad[:, :, 1:15, :], op=OP.add)
            p3cv = p3c[:].rearrange("p a h w -> p a (h w)")

            # A3 swapped channel matmul: p3T[pix, ch] (2 pixel chunks of 98)
            p3tb = sb.tile([101, 448], BF)
            nc.sync.dma_start(p3tb[98:101, :], dt["coefw"].ap())
            p3ta = sb.tile([98, 448], BF)
            for m0, dst in [(0, p3ta), (98, p3tb)]:
                pst = psA.tile([98, 448], F32, tag="a1")
                for kc in range(8):
                    nc.tensor.matmul(pst[:], p3cv[:, kc, m0:m0 + 98],
                                     sw3[:, kc, :], start=(kc == 0), stop=(kc == 7))
                nc.scalar.copy(dst[0:98, :], pst[:])

            # up4(P3)+K0 staging and A1, software-pipelined per chunk so
            # the ACT copy queue alternates between the two streams
            def emit_u3k0(cc):
                ccs = slice(cc * 112, (cc + 1) * 112)
                for o, wd in nchunks(HW, 448):
                    psu = psU.tile([112, 448], F32, tag="u3")
                    nc.tensor.matmul(psu[:, 0:wd], p3ta[:, ccs], s3a[:, o:o + wd],
                                     start=True, stop=False)
                    nc.tensor.matmul(psu[:, 0:wd], p3tb[:, ccs], s3b[:, o:o + wd],
                                     start=False, stop=True)
                    nc.scalar.copy(u3k0[:, cc, o:o + wd], psu[:, 0:wd])

            phi2s = []
            for cc in range(4):
                emit_u3k0(cc)
            for cc in range(4):
                ccs = slice(cc * 112, (cc + 1) * 112)
                phi = phi_cc(cc)
                build_a1(nc, sb2, psA, sp1, sw1, ccs, phi,
                         u3k0[:, cc, :].rearrange("p (a b) -> p a b", b=56))
                ufv = ufs[cc][:].rearrange("p a b c -> p (a b c)")
                for o0 in (0, 1568):  # phi += 0.5625 * up2(pool(A2)), halved
                    nc.vector.scalar_tensor_tensor(
                        phi[:, o0:o0 + 1568], ufv[:, o0:o0 + 1568], 0.5625,
                        phi[:, o0:o0 + 1568], op0=OP.mult, op1=OP.add)
                if 'F' in STAGES:
                    phi2 = sb.tile([112, HW], BF, tag=f"big{cc}")
                    nc.scalar.activation(phi2[:], phi, AF.Square)
                    phi2s.append(phi2)

        if 'F' not in STAGES:
            nc.gpsimd.dma_start(score_d.ap()[0:112, :], phi012[0:112, 0, 0:NRT])
            return

        featcol = sb.tile([128, NRT], F32)
        nc.vector.memset(featcol[:], 3.0)   # pad lanes of the last row tile
        onesb = sb.tile([112, 1], BF)
        nc.vector.memset(onesb[:], 1.0)

        # ================= cdist + top-8 (+ features interleaved) ===========
        bigv = sb.tile([128, NRT * 8], F32)
        nc.vector.memset(bigv[:], -1.0)
        with tc.tile_pool(name="psC", bufs=2, space="PSUM") as psC:
            for rt in range(NRT):
                M = 64 if rt == NRT - 1 else 128
                rts = slice(rt * 128, rt * 128 + M)
                t8 = sb2.tile([128, 24], F32, tag="t8")
                for hi, (hoff, hwid) in enumerate(PIECES):
                    ps = psC.tile([128, 1088], F32, tag="h")
                    # kc3 emitted after kc0-2 of the whole piece: its phi3
                    # dependency resolves last, so don't head-of-line block PE
                    for kcs in ((0, 1, 2), (3,)):
                        for j, (o, wd) in enumerate(nchunks(hwid, 512)):
                            for kc in kcs:
                                lhsT = phi012[:, kc, rts] if kc < 3 else phi3[:, rts]
                                rhs = (rhs012[:, kc, hoff + o:hoff + o + wd] if kc < 3
                                       else rhs3[:, hoff + o:hoff + o + wd])
                                nc.tensor.matmul(ps[0:M, o:o + wd], lhsT, rhs,
                                                 start=(kc == 0), stop=(kc == 3))
                    nc.vector.max(out=t8[0:M, hi * 8:hi * 8 + 8], in_=ps[0:M, 0:hwid])
                nc.vector.max(out=bigv[0:M, rt * 8:rt * 8 + 8], in_=t8[0:M, :])
                if rt == 20 and 'S' in STAGES:
                    emit_score(nc, sb, bigv, featcol, score_d, 0, 21)
                if rt == 1 and 'F' in STAGES:
                    # |phi_s|^2 straight into [128, 25] layout: per row tile,
                    # an N=1 matmul with lhsT=phi^2 columns and rhs=ones sums
                    # the channel axis; tucked behind the first cdist tiles
                    psf = psC.tile([128, NRT], F32, tag="feat")
                    for frt in range(NRT):
                        fM = 64 if frt == NRT - 1 else 128
                        frts = slice(frt * 128, frt * 128 + fM)
                        for cc in range(4):
                            nc.tensor.matmul(psf[0:fM, frt:frt + 1],
                                             phi2s[cc][:, frts], onesb[:],
                                             start=(cc == 0), stop=(cc == 3))
                    nc.scalar.copy(featcol[:, 0:NRT - 1], psf[:, 0:NRT - 1])
                    nc.scalar.copy(featcol[0:64, NRT - 1:NRT], psf[0:64, NRT - 1:NRT])

        if 'C' not in STAGES:
            nc.sync.dma_start(score_d.ap(), featcol[:])
            return
        if 'S' not in STAGES:
            nc.sync.dma_start(score_d.ap(), bigv[:, 0:NRT])
            return
        emit_score(nc, sb, bigv, featcol, score_d, 21, NRT)


def emit_score(nc, sb, bigv, featcol, score_d, t0, t1):
    """score = d0/(1+e^(d0-d1)+e^(d0-d2)) with d_i = sqrt(feat - v_i), for
    row tiles [t0, t1). Batched so only a small slice trails the last tile."""
    nb = t1 - t0
    bv = bigv[:].rearrange("p (t e) -> p t e", e=8)[:, t0:t1, :]
    fc = featcol[:, t0:t1]
    d2b = sb.tile([128, 3, NRT], F32, tag=f"d2b{t0}")
    for i in range(3):
        nc.vector.tensor_tensor(d2b[:, i, 0:nb], fc, bv[:, :, i], op=OP.subtract)
    d3 = sb.tile([128, 3, NRT], F32, tag=f"d3{t0}")
    nc.scalar.activation(d3[:, :, 0:nb], d2b[:, :, 0:nb], AF.Sqrt)
    dif = sb.tile([128, 2, NRT], F32, tag=f"dif{t0}")
    nc.vector.tensor_tensor(dif[:, 0, 0:nb], d3[:, 0, 0:nb], d3[:, 1, 0:nb], op=OP.subtract)
    nc.vector.tensor_tensor(dif[:, 1, 0:nb], d3[:, 0, 0:nb], d3[:, 2, 0:nb], op=OP.subtract)
    ex = sb.tile([128, 2, NRT], F32, tag=f"ex{t0}")
    nc.scalar.activation(ex[:, :, 0:nb], dif[:, :, 0:nb], AF.Exp)
    den = sb.tile([128, NRT], F32, tag=f"den{t0}")
    nc.vector.scalar_tensor_tensor(den[:, 0:nb], ex[:, 0, 0:nb], 1.0, ex[:, 1, 0:nb],
                                   op0=OP.add, op1=OP.add)
    rec = sb.tile([128, NRT], F32, tag=f"rec{t0}")
    nc.vector.reciprocal(rec[:, 0:nb], den[:, 0:nb])
    sco = sb.tile([128, NRT], F32, tag=f"sco{t0}")
    nc.vector.tensor_tensor(sco[:, 0:nb], rec[:, 0:nb], d3[:, 0, 0:nb], op=OP.mult)
    nc.sync.dma_start(score_d.ap()[:, t0:t1], sco[:, 0:nb])


def build_a1(nc, sb2, psA, sp1, sw1, ccs, phi, u3v):
    """A1 = W1*p1 with PE-fused w-pool; h-pool accumulates into u3v and the
    final add writes phi (initializing it)."""
    t1pad = sb2.tile([112, 58, 56], BF, tag="t1pad")
    nc.vector.memset(t1pad[:, 0, :], 0.0)
    nc.vector.memset(t1pad[:, 57, :], 0.0)
    for g in range(7):  # 8 h-rows per psum tile
        ps = psA.tile([112, 8, 56], F32, tag="a1")
        first = True
        for kc in range(2):
            rv = sp1[:, kc, :].rearrange("p (h w) -> p h w", w=58)
            for s in range(3):
                nc.tensor.matmul(ps[:], sw1[:, kc, ccs],
                                 rv[:, g * 8:(g + 1) * 8, s:s + 56],
                                 start=first, stop=(kc == 1 and s == 2))
                first = False
        nc.scalar.copy(t1pad[:, 1 + g * 8:1 + (g + 1) * 8, :], ps[:])
    phiv = phi.rearrange("p (a b) -> p a b", b=56)
    for r0, r1 in ((0, 28), (28, 56)):
        nc.vector.tensor_tensor(u3v[:, r0:r1, :], u3v[:, r0:r1, :],
                                t1pad[:, r0:r1, :], op=OP.add)
        nc.vector.tensor_tensor(u3v[:, r0:r1, :], u3v[:, r0:r1, :],
                                t1pad[:, r0 + 2:r1 + 2, :], op=OP.add)
        nc.vector.tensor_tensor(phiv[:, r0:r1, :], u3v[:, r0:r1, :],
                                t1pad[:, r0 + 1:r1 + 1, :], op=OP.add)


def build_a2_r2(nc, sb, sb1, sb2, psA, sp2, sw2, ccs, cc, ve=None):
    """A2 = W2*p2 (PE w-pool) -> h-pool -> 2x bilinear phases -> returns uf.

    ve may be nc.gpsimd: it gets only tensor_tensor adds (the only elementwise
    op walrus accepts on Pool); scaled ops stay on DVE as prescales/edges.
    The final phi += 0.5625*uf is emitted by the caller once phi exists."""
    if ve is None:
        ve = nc.vector
    t2pad = sb2.tile([112, 30, 28], BF, tag="t2pad")
    nc.vector.memset(t2pad[:, 0, :], 0.0)
    nc.vector.memset(t2pad[:, 29, :], 0.0)
    for r0, nr in [(0, 16), (16, 12)]:
        ps = psA.tile([112, 16, 28], F32, tag="a2")
        first = True
        for kc in range(4):
            rv = sp2[:, kc, :].rearrange("p (h w) -> p h w", w=30)
            for s in range(3):
                nc.tensor.matmul(ps[:, 0:nr, :], sw2[:, kc, ccs],
                                 rv[:, r0:r0 + nr, s:s + 28],
                                 start=first, stop=(kc == 3 and s == 2))
                first = False
        nc.scalar.copy(t2pad[:, 1 + r0:1 + r0 + nr, :], ps[:, 0:nr, :])
    p2t = sb2.tile([112, 28, 28], BF, tag="p2t")
    ve.tensor_tensor(p2t[:], t2pad[:, 0:28, :], t2pad[:, 2:30, :], op=OP.add)
    ve.tensor_tensor(p2t[:], p2t[:], t2pad[:, 1:29, :], op=OP.add)

    # R2: separable 2x upsample; per-axis scale 0.75 deferred (0.5625 at the
    # end). Far taps are prescaled by 1/3 on DVE so the phases are pure adds.
    p2t3 = sb1.tile([112, 28, 28], BF, tag="p2t3")
    nc.vector.tensor_scalar(p2t3[:], p2t[:], 1.0 / 3.0, None, op0=OP.mult)
    u2w = sb2.tile([112, 28, 28, 2], BF, tag="u2w")   # [.., t, parity]
    ve.tensor_tensor(u2w[:, :, 1:28, 0], p2t3[:, :, 0:27], p2t[:, :, 1:28], op=OP.add)
    ve.tensor_tensor(u2w[:, :, 0:27, 1], p2t3[:, :, 1:28], p2t[:, :, 0:27], op=OP.add)
    nc.vector.tensor_scalar(u2w[:, :, 0, 0], p2t[:, :, 0], 4.0 / 3.0, None, op0=OP.mult)
    nc.vector.tensor_scalar(u2w[:, :, 27, 1], p2t[:, :, 27], 4.0 / 3.0, None, op0=OP.mult)
    u2wv = u2w[:].rearrange("p h t e -> p h (t e)")   # [112, 28, 56]
    u2w3 = sb1.tile([112, 28, 56], BF, tag="u2w3")
    nc.vector.tensor_scalar(u2w3[:], u2wv, 1.0 / 3.0, None, op0=OP.mult)
    uf = sb.tile([112, 28, 2, 56], BF, tag=f"big{cc}")  # [.., s, parity, w]
    ve.tensor_tensor(uf[:, 1:28, 0, :], u2w3[:, 0:27, :], u2wv[:, 1:28, :], op=OP.add)
    ve.tensor_tensor(uf[:, 0:27, 1, :], u2w3[:, 1:28, :], u2wv[:, 0:27, :], op=OP.add)
    nc.vector.tensor_scalar(uf[:, 0, 0, :], u2wv[:, 0, :], 4.0 / 3.0, None, op0=OP.mult)
    nc.vector.tensor_scalar(uf[:, 27, 1, :], u2wv[:, 27, :], 4.0 / 3.0, None, op0=OP.mult)
    return uf


def nchunks(total, step):
    out = []
    o = 0
    while o < total:
        out.append((o, min(step, total - o)))
        o += step
    return out


# ---------------------------------------------------------------- entrypoint
_CACHE = {}


def kernel(p1, p2, p3, W, b, C):
    """Full-input entrypoint: shards batch B=8 across 8 NeuronCores (one
    image per core), runs the Bass kernel, gathers to [8, 1, 56, 56] f32."""
    from concourse import bass_utils
    B = np.asarray(p1).shape[0]
    n_cores = 8
    assert B == n_cores, f"expected batch 8, got {B}"
    if 'nc' not in _CACHE:
        _CACHE['nc'] = build_nc(n_cores=n_cores)
    nc = _CACHE['nc']
    shared = host_prep_shared(W, b, C)
    in_maps = []
    for i in range(B):
        core = host_prep_core(p1[i], p2[i], p3[i])
        in_maps.append({**shared, **core})
    res = bass_utils.run_bass_kernel_spmd(nc, in_maps,
                                          core_ids=list(range(n_cores)))
    out = np.stack([host_postprocess(res.results[i]["score"])
                    for i in range(B)]).astype(np.float32)
    return out[:, None, :, :]

